# revision 1
# baseline (speedup 1.0000x reference)
# Trainium2 Bass kernel for CubeDiagonalAttention.
#
# reference math:
#   z = x @ W.T                         [B, N, 3]
#   s = sign(z)                         (+-1 a.s.)
#   hamming[i,j] = sum_k (s_i,k != s_j,k)
#   bias[i,j] = diag_weights[hamming[i,j]]
#
# Kernel identity (exact): with c_i the 3-bit sign code of row i and
# chi_S(c) = prod_{k in S} s_k the 8 cube characters,
#   bias[i,j] = sum_S (lam_S / 8) chi_S(c_i) chi_S(c_j)
# where lam_S = sum_e diag_weights[popcount(e)] * (-1)^{popcount(S & e)}
# is the eigenvalue of the distance-weight matrix on the hypercube.
# So bias = (Lam * F_q)^T-style K=8 matmul of +-1 character features.
# chi values are +-1 (exact in bf16); for the given diag_weights lam/8
# is exact in bf16 and PSUM f32 accumulation of 8 exact terms is exact,
# so the kernel output matches the reference bit-for-bit given equal
# signs of z (margin: min |z| ~ 2e-5 >> f32 matmul rounding ~1e-6).
#
# Sharding (8 cores): core c -> batch b = c // 2, query-half h = c % 2.
# Each core receives x[b] rolled by -h*2048 rows, computes signs for all
# 4096 rows (keys), uses rows 0:2048 as queries, and emits a [2048, 4096]
# row-block whose columns the host un-rolls.

import sys

import numpy as np

P = 128
B = 4
N = 4096
D = 1024
NQ = 2048
CC = 512  # output column chunk (one PSUM bank of f32)


def _import_concourse():
    try:
        import concourse.bass  # noqa: F401
    except ImportError:
        for p in ("/opt/trn_rl_repo", "/root/.axon_site/_ro/trn_rl_repo"):
            if p not in sys.path:
                sys.path.insert(0, p)
        import concourse.bass  # noqa: F401


def build_program(n=N, d=D, nq=NQ, out_dt="fp8", ow=2):
    """Emit the SPMD per-core program. Parameterized so a scaled-down
    version can run under CoreSim. out_dt: the bias values are the four
    diag_weights themselves; when those are exactly representable in a
    narrow dtype ("fp8" e4m3 / "bf16") the output tensor is written
    narrow (1/4 resp. 1/2 the DMA-write traffic) and upcast on host."""
    _import_concourse()
    from contextlib import ExitStack

    import concourse.mybir as mybir
    import concourse.tile as tile
    from concourse import bacc
    from concourse.masks import make_identity

    f32 = mybir.dt.float32
    bf16 = mybir.dt.bfloat16

    nt = n // P  # key row tiles
    ndc = d // P  # contraction chunks
    nqt = nq // P  # query row tiles
    ncc = n // CC  # output column chunks

    odt = {"fp8": mybir.dt.float8e4, "bf16": bf16, "f32": f32}[out_dt]
    nc = bacc.Bacc()
    xb = nc.declare_dram_parameter("xb", [n, d], f32, isOutput=False)
    wt = nc.declare_dram_parameter("wt", [d, 3], f32, isOutput=False)
    lam = nc.declare_dram_parameter("lam", [8, 1], f32, isOutput=False)
    out = nc.declare_dram_parameter("out", [nq, n], odt, isOutput=True)

    # phase-3 work unit = (cc group, rt): OW FT chunks feed one output DMA;
    # ready once those FT chunks (key tiles) and the UFT quad-chunk
    # holding rt are both written
    OW = ow
    ngrp = max(ncc // OW, 1)
    ready = {}
    for ccp in range(ngrp):
        last_tile = min((OW * ccp + OW) * (CC // P) - 1, nt - 1)
        for rt in range(nqt):
            rt_ready = min(4 * (rt // 4) + 3, nt - 1)
            ready.setdefault(max(last_tile, rt_ready), []).append((ccp, rt))

    with tile.TileContext(nc) as tc, ExitStack() as ctx:
        const = ctx.enter_context(tc.tile_pool(name="const", bufs=1))
        ident = const.tile([P, P], f32, name="ident")
        make_identity(nc, ident)
        wt_sb = const.tile([P, ndc, 3], f32, name="wt_sb")
        nc.sync.dma_start(out=wt_sb, in_=wt.rearrange("(c p) k -> p c k", p=P))
        lam_sb = const.tile([8, 1], f32, name="lam_sb")
        nc.sync.dma_start(out=lam_sb, in_=lam[:, :])

        # character matrices, bf16: FT[cc] = chi rows for key columns of
        # quad cc, UFTC[qc] = (lam/8)-weighted chi for query quad qc
        QD = CC // P  # tiles per quad / per FT chunk
        nquad = nt // QD
        nqq = (nqt + QD - 1) // QD
        GT = min(4, ndc)  # transposes per PSUM-bank group
        ft = [const.tile([8, CC], bf16, name=f"ft{i}") for i in range(ncc)]
        uftc = [const.tile([8, CC], bf16, name=f"uftc{i}") for i in range(nqq)]

        xpool = ctx.enter_context(tc.tile_pool(name="xpool", bufs=4))
        xtpool = ctx.enter_context(tc.tile_pool(name="xtpool", bufs=6))
        fpool = ctx.enter_context(tc.tile_pool(name="fpool", bufs=4))
        opool = ctx.enter_context(tc.tile_pool(name="opool", bufs=8))
        ppool = ctx.enter_context(tc.tile_pool(name="ppool", bufs=3, space="PSUM"))
        zpool = ctx.enter_context(tc.tile_pool(name="zpool", bufs=2, space="PSUM"))
        opsum = ctx.enter_context(tc.tile_pool(name="opsum", bufs=3, space="PSUM"))

        n_out_copies = 0
        n_xt_copies = 0
        for q in range(nquad):
            fquad = fpool.tile([P, QD, 8], f32, name="fquad", tag="fquad")
            tf = ppool.tile([P, CC], f32, name="tf", tag="tp")
            for half in range(QD // 2):
                # x loaded two row-tiles per DMA (1 MiB transfers)
                t0 = q * QD + 2 * half
                xtile2 = xpool.tile([P, 2, d], f32, name="xtile2", tag="x2")
                nc.sync.dma_start(
                    out=xtile2,
                    in_=xb[t0 * P : (t0 + 2) * P, :].rearrange(
                        "(two p) d -> p two d", p=P
                    ),
                )
                for sub in range(2):
                    tq = 2 * half + sub  # tile index within quad
                    xtile = xtile2[:, sub, :]
                    nc.gpsimd.memset(fquad[:, tq, 0:1], 1.0)
                    xts = []
                    for g in range(ndc // GT):  # transpose groups
                        tp = ppool.tile([P, GT * P], f32, name="tp", tag="tp")
                        for j in range(GT):
                            dc = GT * g + j
                            nc.tensor.transpose(
                                tp[:, j * P : (j + 1) * P],
                                xtile[:, dc * P : (dc + 1) * P],
                                ident,
                            )
                        xt = xtpool.tile([P, GT * P], f32, name="xt", tag="xt")
                        if n_xt_copies % 2 == 0:
                            nc.vector.tensor_copy(xt, tp)
                        else:
                            nc.scalar.copy(xt, tp)
                        n_xt_copies += 1
                        xts.append(xt)
                    zp = zpool.tile([P, 3], f32, name="zp", tag="zp")
                    for dc in range(ndc):
                        nc.tensor.matmul(
                            zp,
                            lhsT=xts[dc // GT][
                                :, (dc % GT) * P : (dc % GT + 1) * P
                            ],
                            rhs=wt_sb[:, dc, :],
                            start=(dc == 0),
                            stop=(dc == ndc - 1),
                        )
                    nc.scalar.sign(fquad[:, tq, 1:4], zp)

            # cube characters for the whole quad (strided over tiles)
            nc.vector.tensor_mul(fquad[:, :, 4:5], fquad[:, :, 1:2], fquad[:, :, 2:3])
            nc.vector.tensor_mul(fquad[:, :, 5:6], fquad[:, :, 1:2], fquad[:, :, 3:4])
            nc.vector.tensor_mul(fquad[:, :, 6:7], fquad[:, :, 2:3], fquad[:, :, 3:4])
            nc.vector.tensor_mul(fquad[:, :, 7:8], fquad[:, :, 4:5], fquad[:, :, 3:4])
            for tq in range(QD):
                nc.tensor.transpose(
                    tf[0:8, tq * P : (tq + 1) * P], fquad[:, tq, :], ident
                )
            nc.vector.tensor_copy(ft[q], tf[0:8, :])
            if q < nqq:
                nc.vector.tensor_scalar_mul(uftc[q], tf[0:8, :], lam_sb)

            # interleaved phase 3: bias chunk = (lam*F_q)^T . F_k, K=8
            t = q * QD + QD - 1
            for ccp, ort in ready.get(t, []):
                ccs = [c for c in range(OW * ccp, OW * ccp + OW) if c < ncc]
                w = len(ccs) * CC
                osb = opool.tile([P, OW * CC], odt, name="osb", tag="osb")
                lhs = uftc[ort // QD][:, (ort % QD) * P : (ort % QD + 1) * P]
                for j, occ in enumerate(ccs):
                    pot = opsum.tile([P, CC], f32, name="pot", tag="pot")
                    nc.tensor.matmul(
                        pot, lhsT=lhs, rhs=ft[occ], start=True, stop=True
                    )
                    if n_out_copies % 2 == 1:
                        nc.scalar.copy(osb[:, j * CC : (j + 1) * CC], pot)
                    else:
                        nc.vector.tensor_copy(osb[:, j * CC : (j + 1) * CC], pot)
                    n_out_copies += 1
                nc.sync.dma_start(
                    out=out[
                        ort * P : (ort + 1) * P,
                        OW * ccp * CC : OW * ccp * CC + w,
                    ],
                    in_=osb[:, :w],
                )

    nc.compile()
    return nc


def _lambda_over_8(diag_weights):
    """lam_S / 8 in character order [1, s1, s2, s3, s1s2, s1s3, s2s3, s1s2s3]
    (subset bitmasks [0, 1, 2, 4, 3, 5, 6, 7])."""
    w = np.asarray(diag_weights, dtype=np.float64)
    lam = np.zeros(8)
    for S in range(8):
        lam[S] = sum(
            w[bin(e).count("1")] * (-1) ** bin(S & e).count("1") for e in range(8)
        ) / 8.0
    order = [0b000, 0b001, 0b010, 0b100, 0b011, 0b101, 0b110, 0b111]
    return lam[order].astype(np.float32).reshape(8, 1)


def kernel(x, W, diag_weights):
    _import_concourse()
    from concourse.bass_utils import run_bass_kernel_spmd

    x = np.ascontiguousarray(np.asarray(x, dtype=np.float32))
    W = np.asarray(W, dtype=np.float32)
    assert x.shape == (B, N, D) and W.shape == (3, D)

    wt = np.ascontiguousarray(W.T)  # [D, 3]
    lam = _lambda_over_8(diag_weights)

    import ml_dtypes

    dw = np.asarray(diag_weights, dtype=np.float32)
    if np.all(dw.astype(ml_dtypes.float8_e4m3).astype(np.float32) == dw):
        out_dt = "fp8"
    elif np.all(dw.astype(ml_dtypes.bfloat16).astype(np.float32) == dw):
        out_dt = "bf16"
    else:
        out_dt = "f32"

    in_maps = []
    for c in range(8):
        b, h = divmod(c, 2)
        xb = x[b] if h == 0 else np.ascontiguousarray(np.roll(x[b], -NQ, axis=0))
        in_maps.append({"xb": xb, "wt": wt, "lam": lam})

    nc = build_program(out_dt=out_dt)
    res = run_bass_kernel_spmd(nc, in_maps, list(range(8))).results

    out = np.empty((B, N, N), dtype=np.float32)
    for c in range(8):
        b, h = divmod(c, 2)
        o = np.asarray(res[c]["out"]).astype(np.float32)
        if h:
            o = np.roll(o, NQ, axis=1)
        out[b, h * NQ : (h + 1) * NQ, :] = o
    return out

